# Initial kernel scaffold
#
"""DeMBR multi-behavior LightGCN kernel for Trainium2 (8 NeuronCores).

v2 strategy (per dense behavior, each [N,N] relation matrix R):
  - Hybrid block sharding. Core k holds TWO fp8(e3m4) views of R:
      At = R[rows 512k:512k+512, :].T   as [128, 32ic, 512u]  (item-partition)
      Ac = R[:, cols 512k:512k+512]     as [128, 32uc, 512j]  (user-partition)
    e3m4 (4 mantissa bits) keeps the R quantization error ~2x below e4m3;
    embeddings and gathered intermediates stay bf16 (mixed-dtype matmuls run
    at bf16 rate on the PE; only fp32 operands must match).
  - With At the user side is fully local (contract over all items); with Ac
    the item side is fully local (contract over all users). The only
    cross-core exchange is an AllGather of the layer-1 [u1|i1] blocks,
    batched into TWO collectives ({behaviors 0..nb-2} up front, {nb-1}
    alone last) because each TOPSP collective carries ~20us of latency on
    top of its transfer: the big group rides the one-time setup window and
    only one 14us P2 group serializes behind the small final gather. This
    replaces the baseline's per-behavior [64,4096] AllReduce + f32 wT
    outputs. (An e4m3 payload was tried and is numerically free, but the
    halved readback line size made it ~20us slower net - keep bf16.)
  - 4 matmul passes per behavior, all with R as the 512-wide moving operand:
      P1u: u1_un.T [64,512] = i0.T @ At      (32 mm, accumulate over items)
      P1i: i1_un.T [64,512] = u0.T @ Ac      (32 mm, accumulate over users)
      P2u: u2_un.T [64,512] = i1_full.T @ At (stationary from gathered buf)
      P2i: i2_un.T [64,512] = u1_full.T @ Ac
    P1 results are PE-transposed to natural [128,4,64], scaled by the host
    degree reciprocals, exported f32, and packed bf16 into the AG payload.
    P2 raw psums are exported [64,512] f32; the host applies deg scaling for
    layer 2 and the /2 averaging during assembly.
  - Schedule: a tiny warmup AllGather on garbage data fires first to absorb
    the cc stream's one-time ~35-60us setup; all At/Ac loads are prefetched
    up front on the two HWDGE rings; ~3.5us of dummy matmuls warm the PE's
    HAM clock to 2.4GHz; then all four P1 groups run back-to-back (covering
    the paired AllGathers), followed by the four P2 groups. Gather readbacks
    ride the by-then-idle sync ring so they never head-block a load.
  - deg_u/deg_i are computed on the host in f64 from the original f32 R
    (exactly matching the reference); all-ones virtual matrices are detected
    and computed analytically.

kernel(**inputs) takes the full unsharded inputs and returns [14, 4096, 64].
"""

import os
import numpy as np
import ml_dtypes

EPS = 1e-8
N, D = 4096, 64
P = 128
NCORES = 8
BLK = N // NCORES           # 512 users/items per core block
NB_CH = BLK // P            # 4 chunks per block
NCH = N // P                # 32 chunks over the full dim

_BF16 = ml_dtypes.bfloat16
_E3M4 = ml_dtypes.float8_e3m4


# --------------------------------------------------------------------------
# device program
# --------------------------------------------------------------------------

def build_program(nb):
    """Build + bacc-compile the SPMD program for `nb` dense behaviors."""
    import concourse.bass as bass  # noqa: F401  (registers types)
    import concourse.mybir as mybir
    import concourse.tile as tile
    from concourse import bacc
    from concourse.masks import make_identity

    f32, bf16 = mybir.dt.float32, mybir.dt.bfloat16
    fp8 = mybir.dt.float8e3
    ALU = mybir.AluOpType

    nc = bacc.Bacc("TRN2", target_bir_lowering=False, debug=False,
                   num_devices=NCORES)

    At_in = [nc.dram_tensor(f"At{b}", [P, NCH, BLK], fp8, kind="ExternalInput")
             for b in range(nb)]
    Ac_in = [nc.dram_tensor(f"Ac{b}", [P, NCH, BLK], fp8, kind="ExternalInput")
             for b in range(nb)]
    i0s_in = nc.dram_tensor("i0s", [P, NCH, D], bf16, kind="ExternalInput")
    u0s_in = nc.dram_tensor("u0s", [P, NCH, D], bf16, kind="ExternalInput")
    # per-behavior reciprocal degree slices for this core's blocks
    rud_in = nc.dram_tensor("rud", [P, nb, NB_CH], f32, kind="ExternalInput")
    rid_in = nc.dram_tensor("rid", [P, nb, NB_CH], f32, kind="ExternalInput")

    u1_out = [nc.dram_tensor(f"u1_{b}", [P, NB_CH, D], f32,
                             kind="ExternalOutput") for b in range(nb)]
    i1_out = [nc.dram_tensor(f"i1_{b}", [P, NB_CH, D], f32,
                             kind="ExternalOutput") for b in range(nb)]
    u2_out = [nc.dram_tensor(f"u2_{b}", [D, BLK], f32,
                             kind="ExternalOutput") for b in range(nb)]
    i2_out = [nc.dram_tensor(f"i2_{b}", [D, BLK], f32,
                             kind="ExternalOutput") for b in range(nb)]
    warm_out = nc.dram_tensor("warm", [P, 2], f32, kind="ExternalOutput")

    rg = [list(range(NCORES))]

    with tile.TileContext(nc) as tc:
        with (
            tc.tile_pool(name="big", bufs=nb) as pbig,
            tc.tile_pool(name="gat", bufs=nb) as pgat,
            tc.tile_pool(name="sm", bufs=4) as psm,
            tc.tile_pool(name="out1", bufs=4) as pout1,
            tc.tile_pool(name="one", bufs=1) as pone,
            tc.tile_pool(name="mm", bufs=4, space="PSUM") as pmm,
            tc.tile_pool(name="tr", bufs=2, space="PSUM") as ptr,
            tc.tile_pool(name="dram", bufs=4, space="DRAM") as pdr,
        ):
            # warmup collective fired immediately (input is an uninitialized
            # DRAM tile - AllGather/bypass only moves bytes): absorbs the
            # one-time ~50us cc-stream setup under the P1 matmul phase
            w_in = pdr.tile([P, 1], f32, tag="w_in", name="w_in")
            w_out = pdr.tile([NCORES, P, 1], f32, tag="w_out", name="w_out",
                             addr_space="Shared")
            nc.gpsimd.collective_compute(
                "AllGather", ALU.bypass, replica_groups=rg,
                ins=[w_in.opt()], outs=[w_out.opt()])

            ident = pone.tile([P, P], f32)
            make_identity(nc, ident[:])

            # ~3.5us of dummy matmuls while the first loads stream: tips the
            # PE's HAM activity window into the 2.4 GHz state before the real
            # work arrives (results exported so DCE keeps them)
            Pw = pmm.tile([P, P], f32, tag="WARM", name="Pw", bufs=1)
            for it in range(28):
                nc.tensor.matmul(Pw[:], ident[:], ident[:],
                                 start=(it == 0), stop=(it == 27))
            warm_sb = pone.tile([P, 2], f32)
            nc.vector.tensor_copy(out=warm_sb[:], in_=Pw[:, 0:2])
            nc.scalar.dma_start(out=warm_out.ap(), in_=warm_sb[:])

            i0s = pone.tile([P, NCH, D], bf16)
            nc.sync.dma_start(out=i0s[:], in_=i0s_in[:])
            u0s = pone.tile([P, NCH, D], bf16)
            nc.scalar.dma_start(out=u0s[:], in_=u0s_in[:])
            rud = pone.tile([P, nb, NB_CH], f32)
            nc.sync.dma_start(out=rud[:], in_=rud_in[:])
            rid = pone.tile([P, nb, NB_CH], f32)
            nc.sync.dma_start(out=rid[:], in_=rid_in[:])

            at_tiles, ac_tiles, state = {}, {}, {}

            def load(b):
                # At on the SP hwdge ring, Ac on the ACT ring: the two
                # physical HWDGE queues stream in parallel. Behavior 0's
                # quarters alternate across both rings so the very first P1
                # matmuls aren't load-starved.
                At = pbig.tile([P, NCH, BLK], fp8, tag="At", name=f"At{b}")
                Ac = pbig.tile([P, NCH, BLK], fp8, tag="Ac", name=f"Ac{b}")
                q = NCH // 4
                for g in range(4):
                    if b == 0:
                        eng_t = nc.sync if g % 2 == 0 else nc.scalar
                        eng_c = nc.scalar if g % 2 == 0 else nc.sync
                    else:
                        eng_t, eng_c = nc.sync, nc.scalar
                    eng_t.dma_start(out=At[:, g * q:(g + 1) * q, :],
                                    in_=At_in[b][:, g * q:(g + 1) * q, :])
                    eng_c.dma_start(out=Ac[:, g * q:(g + 1) * q, :],
                                    in_=Ac_in[b][:, g * q:(g + 1) * q, :])
                at_tiles[b], ac_tiles[b] = At, Ac

            def phase1(b):
                At, Ac = at_tiles[b], ac_tiles[b]
                # ---- P1u: u1_un.T = i0.T @ At ; P1i: i1_un.T = u0.T @ Ac
                Pu = pmm.tile([D, BLK], f32, tag="PC", name=f"P1u_{b}")
                for c in range(NCH):
                    nc.tensor.matmul(Pu[:], i0s[:, c, :], At[:, c, :],
                                     start=(c == 0), stop=(c == NCH - 1))
                Su = psm.tile([D, BLK], f32, tag="S", name=f"Su{b}")
                nc.vector.tensor_copy(out=Su[:], in_=Pu[:])

                Pi = pmm.tile([D, BLK], f32, tag="PC", name=f"P1i_{b}")
                for c in range(NCH):
                    nc.tensor.matmul(Pi[:], u0s[:, c, :], Ac[:, c, :],
                                     start=(c == 0), stop=(c == NCH - 1))
                Si = psm.tile([D, BLK], f32, tag="S", name=f"Si{b}")
                nc.vector.tensor_copy(out=Si[:], in_=Pi[:])

                # ---- transpose to natural layout + deg scale
                PT = ptr.tile([P, 2 * NB_CH, D], f32, tag="PT", name=f"PT{b}")
                for f in range(NB_CH):
                    nc.tensor.transpose(PT[:, f, :],
                                        Su[:, f * P:(f + 1) * P],
                                        ident[0:D, 0:D])
                for f in range(NB_CH):
                    nc.tensor.transpose(PT[:, NB_CH + f, :],
                                        Si[:, f * P:(f + 1) * P],
                                        ident[0:D, 0:D])
                u1s = pout1.tile([P, NB_CH, D], f32, tag="u1s", name=f"u1s{b}")
                i1s = pout1.tile([P, NB_CH, D], f32, tag="i1s", name=f"i1s{b}")
                zi = psm.tile([P, NB_CH, 2 * D], bf16, tag="zi", name=f"zi{b}")
                for f in range(NB_CH):
                    nc.vector.tensor_scalar_mul(out=u1s[:, f, :],
                                                in0=PT[:, f, :],
                                                scalar1=rud[:, b, f:f + 1])
                    nc.vector.tensor_scalar_mul(out=i1s[:, f, :],
                                                in0=PT[:, NB_CH + f, :],
                                                scalar1=rid[:, b, f:f + 1])
                nc.vector.tensor_copy(out=zi[:, :, 0:D], in_=u1s[:])
                nc.vector.tensor_copy(out=zi[:, :, D:2 * D], in_=i1s[:])
                nc.scalar.dma_start(out=u1_out[b].ap(), in_=u1s[:])
                nc.scalar.dma_start(out=i1_out[b].ap(), in_=i1s[:])

                # write this behavior's payload into its group's AG input
                gi, v = grp_of[b]
                z_in = zin_groups[gi]
                nc.scalar.dma_start(out=z_in[:, v, :, :], in_=zi[:])
                state[b] = None

            def gather_group(gi):
                # batched AllGather: front group carries nb-1 behaviors, the
                # last behavior gathers alone, so only one 14us P2 group
                # serializes behind the final (small) collective while the
                # per-op ~20us TOPSP latency is paid as few times as possible
                members = groups[gi]
                z_in = zin_groups[gi]
                z_out = pdr.tile([NCORES, P, len(members), NB_CH, 2 * D],
                                 bf16, tag=f"z_out{gi}", name=f"z_out{gi}",
                                 addr_space="Shared")
                nc.gpsimd.collective_compute(
                    "AllGather", ALU.bypass, replica_groups=rg,
                    ins=[z_in.opt()], outs=[z_out.opt()])
                for v, b in enumerate(members):
                    G = pgat.tile([P, NCORES, NB_CH, 2 * D], bf16, tag="G",
                                  name=f"G{b}")
                    nc.sync.dma_start(
                        out=G[:],
                        in_=z_out[:, :, v, :, :].rearrange(
                            "c p f x -> p c f x"))
                    state[b] = G

            def phase2(b):
                At, Ac = at_tiles.pop(b), ac_tiles.pop(b)
                G = state.pop(b)
                # ---- P2u: u2_un.T = i1_full.T @ At
                Pu = pmm.tile([D, BLK], f32, tag="PC", name=f"P2u_{b}")
                for c in range(NCH):
                    nc.tensor.matmul(Pu[:], G[:, c // NB_CH, c % NB_CH, D:2 * D],
                                     At[:, c, :],
                                     start=(c == 0), stop=(c == NCH - 1))
                Tu = psm.tile([D, BLK], f32, tag="T2", name=f"Tu{b}")
                nc.vector.tensor_copy(out=Tu[:], in_=Pu[:])
                nc.scalar.dma_start(out=u2_out[b].ap(), in_=Tu[:])
                # ---- P2i: i2_un.T = u1_full.T @ Ac
                Pi = pmm.tile([D, BLK], f32, tag="PC", name=f"P2i_{b}")
                for c in range(NCH):
                    nc.tensor.matmul(Pi[:], G[:, c // NB_CH, c % NB_CH, 0:D],
                                     Ac[:, c, :],
                                     start=(c == 0), stop=(c == NCH - 1))
                Ti = psm.tile([D, BLK], f32, tag="T2", name=f"Ti{b}")
                nc.vector.tensor_copy(out=Ti[:], in_=Pi[:])
                nc.scalar.dma_start(out=i2_out[b].ap(), in_=Ti[:])

            # all loads upfront (HWDGE rings drain in ~30us, long before the
            # gather readbacks queue behind them); all P1s before all P2s so
            # every AllGather hides under the remaining P1 matmul stream
            if nb >= 2:
                groups = [list(range(nb - 1)), [nb - 1]]
            else:
                groups = [list(range(nb))]
            grp_of = {}
            for gi, members in enumerate(groups):
                for v, b in enumerate(members):
                    grp_of[b] = (gi, v)
            zin_groups = [
                pdr.tile([P, len(members), NB_CH, 2 * D], bf16,
                         tag=f"z_in{gi}", name=f"z_in{gi}")
                for gi, members in enumerate(groups)]
            last_of_group = {members[-1]: gi for gi, members in
                             enumerate(groups)}
            for b in range(nb):
                load(b)
            for b in range(nb):
                phase1(b)
                if b in last_of_group:
                    gather_group(last_of_group[b])
            for b in range(nb):
                phase2(b)

    nc.compile()
    return nc


# --------------------------------------------------------------------------
# host-side helpers
# --------------------------------------------------------------------------

def _chunk_part(x):
    """[4096, C] -> [128, 32, C] with row = c*128 + p."""
    return np.ascontiguousarray(
        x.reshape(NCH, P, x.shape[1]).transpose(1, 0, 2))


def host_prep_behavior(R):
    """Quantize to e3m4 + exact f64 degree reciprocals."""
    Rq = R.astype(_E3M4)
    deg_u = R.sum(axis=1, dtype=np.float64)
    deg_i = R.sum(axis=0, dtype=np.float64)
    ru = (1.0 / (deg_u + EPS)).astype(np.float32)
    ri = (1.0 / (deg_i + EPS)).astype(np.float32)
    return Rq, ru, ri, deg_u, deg_i


def _chunk_order(k):
    """Gather slots arrive in rank order: identity chunk order."""
    return list(range(NCH))


def _core_layouts(Rq, k, order):
    """Per-core At/Ac tiles in [128, 32, 512] linear-DMA order, with the
    32 contraction chunks permuted to the gather slot order."""
    rows = Rq[k * BLK:(k + 1) * BLK, :]          # [512, 4096]
    # At[p, ic, u] = rows[u, ic*128+p]
    At = np.ascontiguousarray(
        rows.T.reshape(NCH, P, BLK)[order].transpose(1, 0, 2))
    cols = Rq[:, k * BLK:(k + 1) * BLK]          # [4096, 512]
    # Ac[p, uc, j] = cols[uc*128+p, j]
    Ac = np.ascontiguousarray(
        cols.reshape(NCH, P, BLK)[order].transpose(1, 0, 2))
    return At, Ac


def prep_in_maps(prepped, u0, i0):
    i0s = _chunk_part(i0.astype(_BF16))
    u0s = _chunk_part(u0.astype(_BF16))
    in_maps = []
    for k in range(NCORES):
        sl = slice(k * BLK, (k + 1) * BLK)
        order = _chunk_order(k)
        m = {"i0s": np.ascontiguousarray(i0s[:, order, :]),
             "u0s": np.ascontiguousarray(u0s[:, order, :])}
        # rud[p, b, f] = 1/deg_u[k*512 + f*128 + p] for behavior b
        m["rud"] = np.ascontiguousarray(np.stack(
            [p[1][sl].reshape(NB_CH, P).T for p in prepped], axis=1))
        m["rid"] = np.ascontiguousarray(np.stack(
            [p[2][sl].reshape(NB_CH, P).T for p in prepped], axis=1))
        for b, p in enumerate(prepped):
            At, Ac = _core_layouts(p[0], k, order)
            m[f"At{b}"] = At
            m[f"Ac{b}"] = Ac
        in_maps.append(m)
    return in_maps


def assemble_dense(results, prepped, nb):
    """Per-behavior (u_acc [N,D], i_acc [N,D]) from per-core outputs."""
    out = []
    for b in range(nb):
        _, ru, ri, _, _ = prepped[b]
        u_acc = np.empty((N, D), np.float32)
        i_acc = np.empty((N, D), np.float32)
        for k in range(NCORES):
            sl = slice(k * BLK, (k + 1) * BLK)
            u1 = results[k][f"u1_{b}"].transpose(1, 0, 2).reshape(BLK, D)
            i1 = results[k][f"i1_{b}"].transpose(1, 0, 2).reshape(BLK, D)
            u2 = results[k][f"u2_{b}"].T * ru[sl][:, None]
            i2 = results[k][f"i2_{b}"].T * ri[sl][:, None]
            u_acc[sl] = (u1 + u2) * np.float32(0.5)
            i_acc[sl] = (i1 + i2) * np.float32(0.5)
        out.append((u_acc, i_acc))
    return out


def ones_behavior(u0, i0):
    """Analytic LightGCN-2-layer outputs when R is all-ones [N, N]."""
    s_i = i0.astype(np.float64).sum(axis=0)
    s_u = u0.astype(np.float64).sum(axis=0)
    d = N + EPS
    u_row = (s_i / d + s_u * N / (d * d)) * 0.5
    i_row = (s_u / d + s_i * N / (d * d)) * 0.5
    u = np.broadcast_to(u_row.astype(np.float32), (N, D)).copy()
    it = np.broadcast_to(i_row.astype(np.float32), (N, D)).copy()
    return u, it


# --------------------------------------------------------------------------
# cached device runner (compile once per behavior-count, run many)
# --------------------------------------------------------------------------

_RUNNERS = {}


class _Runner:
    def __init__(self, nb):
        self.nb = nb
        self.nc = build_program(nb)
        self._jitted = None
        self._meta = None

    def _prep_jit(self):
        import jax
        import numpy as _np
        from jax.sharding import Mesh, PartitionSpec
        from jax.experimental.shard_map import shard_map
        from concourse import bass2jax
        from concourse.bass2jax import _bass_exec_p, partition_id_tensor
        import concourse.mybir as mybir

        bass2jax.install_neuronx_cc_hook()
        nc = self.nc
        partition_name = (nc.partition_id_tensor.name
                          if nc.partition_id_tensor else None)
        in_names, out_names, out_avals, zero_shapes = [], [], [], []
        for alloc in nc.m.functions[0].allocations:
            if not isinstance(alloc, mybir.MemoryLocationSet):
                continue
            name = alloc.memorylocations[0].name
            if alloc.kind == "ExternalInput":
                if name != partition_name:
                    in_names.append(name)
            elif alloc.kind == "ExternalOutput":
                shape = tuple(alloc.tensor_shape)
                dtype = mybir.dt.np(alloc.dtype)
                out_names.append(name)
                out_avals.append(jax.core.ShapedArray(shape, dtype))
                zero_shapes.append((shape, dtype))
        n_params = len(in_names)
        full_in_names = list(in_names) + list(out_names)
        if partition_name is not None:
            full_in_names.append(partition_name)

        def _body(*args):
            operands = list(args)
            if partition_name is not None:
                operands.append(partition_id_tensor())
            outs = _bass_exec_p.bind(
                *operands,
                out_avals=tuple(out_avals),
                in_names=tuple(full_in_names),
                out_names=tuple(out_names),
                lowering_input_output_aliases=(),
                sim_require_finite=True,
                sim_require_nnan=True,
                nc=nc,
            )
            return tuple(outs)

        devices = jax.devices()[:NCORES]
        mesh = Mesh(_np.asarray(devices), ("core",))
        n_outs = len(out_names)
        in_specs = (PartitionSpec("core"),) * (n_params + n_outs)
        out_specs = (PartitionSpec("core"),) * n_outs
        donate = tuple(range(n_params, n_params + n_outs))
        self._jitted = jax.jit(
            shard_map(_body, mesh=mesh, in_specs=in_specs,
                      out_specs=out_specs, check_rep=False),
            donate_argnums=donate, keep_unused=True)
        self._meta = (in_names, out_names, out_avals, zero_shapes, n_params)

    def run(self, in_maps):
        if self._jitted is None:
            self._prep_jit()
        import numpy as _np
        in_names, out_names, out_avals, zero_shapes, n_params = self._meta
        concat_in = [
            _np.concatenate([_np.asarray(in_maps[c][nm]) for c in range(NCORES)],
                            axis=0)
            for nm in in_names]
        concat_zeros = [_np.zeros((NCORES * s[0], *s[1:]), dt)
                        for (s, dt) in zero_shapes]
        out_arrs = self._jitted(*concat_in, *concat_zeros)
        results = []
        for c in range(NCORES):
            results.append({
                nm: _np.asarray(out_arrs[i]).reshape(
                    NCORES, *out_avals[i].shape)[c]
                for i, nm in enumerate(out_names)})
        return results

    def run_traced(self, in_maps, tmpdir=None):
        """Run through run_bass_kernel_spmd with NTFF tracing (recompiles)."""
        _install_trace_shims()
        from concourse.bass_utils import run_bass_kernel_spmd
        return run_bass_kernel_spmd(self.nc, in_maps,
                                    core_ids=list(range(NCORES)),
                                    trace=True, tmpdir=tmpdir)


def _install_trace_shims():
    """This image's antenv lacks axon_hooks (the NTFF-hook registry) and has
    no artifact bucket; recreate the hook from the boot recipe and make
    artifact upload a local no-op."""
    import sys, types, importlib.util

    if "antenv.axon_hooks" not in sys.modules:
        mod = types.ModuleType("antenv.axon_hooks")
        mod._hook = None

        def set_axon_ntff_profile_hook(h):
            mod._hook = h

        def get_axon_ntff_profile_hook():
            return mod._hook

        mod.set_axon_ntff_profile_hook = set_axon_ntff_profile_hook
        mod.get_axon_ntff_profile_hook = get_axon_ntff_profile_hook
        import antenv
        sys.modules["antenv.axon_hooks"] = mod
        antenv.axon_hooks = mod

        spec = importlib.util.spec_from_file_location(
            "trn_boot_shim", "/root/.axon_site/trn_agent_boot/trn_boot.py")
        boot = importlib.util.module_from_spec(spec)
        spec.loader.exec_module(boot)
        hook = boot._ntff_profile_via_ctypes("/opt/axon/libaxon_pjrt.so")
        mod._hook = hook

    import concourse.bass_utils as bu
    if not getattr(bu.upload_artifacts, "_is_local_shim", False):
        def _local_upload(tmpdir):
            return tmpdir
        _local_upload._is_local_shim = True
        bu.upload_artifacts = _local_upload


def get_runner(nb):
    if nb not in _RUNNERS:
        _RUNNERS[nb] = _Runner(nb)
    return _RUNNERS[nb]


# --------------------------------------------------------------------------
# entry point
# --------------------------------------------------------------------------

def _is_ones(a):
    return a[0, 0] == 1.0 and bool(np.all(a == np.float32(1.0)))


def kernel(**inputs):
    inputs = {k: np.asarray(v) for k, v in inputs.items()}
    u0 = np.ascontiguousarray(inputs["user_embedding"], dtype=np.float32)
    i0 = np.ascontiguousarray(inputs["item_embedding"], dtype=np.float32)

    real_names = ["R_click", "R_fav", "R_cart", "R_buy"]
    virt_names = [("M_click", "add_click"), ("M_fav", "add_fav"),
                  ("M_cart", "add_cart")]
    mats = [np.asarray(inputs[n], dtype=np.float32) for n in real_names]
    mats += [np.asarray(inputs[m], dtype=np.float32) for m, _ in virt_names]

    dense_idx = [j for j, a in enumerate(mats) if not _is_ones(a)]
    per_behavior = [None] * 7

    if dense_idx:
        nb = len(dense_idx)
        runner = get_runner(nb)
        prepped = [host_prep_behavior(mats[j]) for j in dense_idx]
        in_maps = prep_in_maps(prepped, u0, i0)
        results = runner.run(in_maps)
        dense = assemble_dense(results, prepped, nb)
        for pos, j in enumerate(dense_idx):
            per_behavior[j] = dense[pos]

    ones_cache = None
    for j, a in enumerate(mats):
        if per_behavior[j] is None:
            if ones_cache is None:
                ones_cache = ones_behavior(u0, i0)
            per_behavior[j] = ones_cache

    ur = [per_behavior[j][0] for j in range(4)]
    ir = [per_behavior[j][1] for j in range(4)]
    uv = [per_behavior[4 + j][0] + np.asarray(inputs[virt_names[j][1]],
                                              dtype=np.float32)
          for j in range(3)]
    iv = [per_behavior[4 + j][1] for j in range(3)]

    out = np.concatenate(
        [np.stack(ur), np.stack(ir), np.stack(uv), np.stack(iv)], axis=0)
    return np.ascontiguousarray(out, dtype=np.float32)



# revision 1
# speedup vs baseline: 7.3900x; 7.3900x over previous
"""DeMBR multi-behavior LightGCN kernel for Trainium2 (8 NeuronCores).

v2 strategy (per dense behavior, each [N,N] relation matrix R):
  - Hybrid block sharding. Core k holds TWO fp8(e3m4) views of R:
      At = R[rows 512k:512k+512, :].T   as [128, 32ic, 512u]  (item-partition)
      Ac = R[:, cols 512k:512k+512]     as [128, 32uc, 512j]  (user-partition)
    e3m4 (4 mantissa bits) keeps the R quantization error ~2x below e4m3;
    embeddings and gathered intermediates stay bf16 (mixed-dtype matmuls run
    at bf16 rate on the PE; only fp32 operands must match).
  - With At the user side is fully local (contract over all items); with Ac
    the item side is fully local (contract over all users). The only
    cross-core exchange is an AllGather of the layer-1 [u1|i1] blocks,
    batched into TWO collectives ({behaviors 0..nb-2} up front, {nb-1}
    alone last) because each TOPSP collective carries ~20us of latency on
    top of its transfer: the big group rides the one-time setup window and
    only one 14us P2 group serializes behind the small final gather. This
    replaces the baseline's per-behavior [64,4096] AllReduce + f32 wT
    outputs. (An e4m3 payload was tried and is numerically free, but the
    halved readback line size made it ~20us slower net - keep bf16.)
  - 4 matmul passes per behavior, all with R as the 512-wide moving operand:
      P1u: u1_un.T [64,512] = i0.T @ At      (32 mm, accumulate over items)
      P1i: i1_un.T [64,512] = u0.T @ Ac      (32 mm, accumulate over users)
      P2u: u2_un.T [64,512] = i1_full.T @ At (stationary from gathered buf)
      P2i: i2_un.T [64,512] = u1_full.T @ Ac
    P1 results are PE-transposed to natural [128,4,64], scaled by the host
    degree reciprocals, exported f32, and packed bf16 into the AG payload.
    P2 raw psums are exported [64,512] f32; the host applies deg scaling for
    layer 2 and the /2 averaging during assembly.
  - Schedule: a tiny warmup AllGather on garbage data fires first to absorb
    the cc stream's one-time ~35-60us setup; all At/Ac loads are prefetched
    up front on the two HWDGE rings; ~3.5us of dummy matmuls warm the PE's
    HAM clock to 2.4GHz; then all four P1 groups run back-to-back (covering
    the paired AllGathers), followed by the four P2 groups. Gather readbacks
    ride the by-then-idle sync ring so they never head-block a load.
  - deg_u/deg_i are computed on the host in f64 from the original f32 R
    (exactly matching the reference); all-ones virtual matrices are detected
    and computed analytically.

kernel(**inputs) takes the full unsharded inputs and returns [14, 4096, 64].
"""

import os
import numpy as np
import ml_dtypes

EPS = 1e-8
N, D = 4096, 64
P = 128
NCORES = 8
BLK = N // NCORES           # 512 users/items per core block
NB_CH = BLK // P            # 4 chunks per block
NCH = N // P                # 32 chunks over the full dim

_BF16 = ml_dtypes.bfloat16
_E3M4 = ml_dtypes.float8_e3m4


# --------------------------------------------------------------------------
# device program
# --------------------------------------------------------------------------

def build_program(nb):
    """Build + bacc-compile the SPMD program for `nb` dense behaviors."""
    import concourse.bass as bass  # noqa: F401  (registers types)
    import concourse.mybir as mybir
    import concourse.tile as tile
    from concourse import bacc
    from concourse.masks import make_identity

    f32, bf16 = mybir.dt.float32, mybir.dt.bfloat16
    fp8 = mybir.dt.float8e3
    ALU = mybir.AluOpType

    nc = bacc.Bacc("TRN2", target_bir_lowering=False, debug=False,
                   num_devices=NCORES)

    At_in = [nc.dram_tensor(f"At{b}", [P, NCH, BLK], fp8, kind="ExternalInput")
             for b in range(nb)]
    Ac_in = [nc.dram_tensor(f"Ac{b}", [P, NCH, BLK], fp8, kind="ExternalInput")
             for b in range(nb)]
    i0s_in = nc.dram_tensor("i0s", [P, NCH, D], bf16, kind="ExternalInput")
    u0s_in = nc.dram_tensor("u0s", [P, NCH, D], bf16, kind="ExternalInput")
    # per-behavior reciprocal degree slices for this core's blocks
    rud_in = nc.dram_tensor("rud", [P, nb, NB_CH], f32, kind="ExternalInput")
    rid_in = nc.dram_tensor("rid", [P, nb, NB_CH], f32, kind="ExternalInput")

    u1_out = [nc.dram_tensor(f"u1_{b}", [P, NB_CH, D], f32,
                             kind="ExternalOutput") for b in range(nb)]
    i1_out = [nc.dram_tensor(f"i1_{b}", [P, NB_CH, D], f32,
                             kind="ExternalOutput") for b in range(nb)]
    u2_out = [nc.dram_tensor(f"u2_{b}", [D, BLK], f32,
                             kind="ExternalOutput") for b in range(nb)]
    i2_out = [nc.dram_tensor(f"i2_{b}", [D, BLK], f32,
                             kind="ExternalOutput") for b in range(nb)]
    warm_out = nc.dram_tensor("warm", [P, 2], f32, kind="ExternalOutput")

    rg = [list(range(NCORES))]

    with tile.TileContext(nc) as tc:
        with (
            tc.tile_pool(name="big", bufs=nb) as pbig,
            tc.tile_pool(name="gat", bufs=nb) as pgat,
            tc.tile_pool(name="sm", bufs=4) as psm,
            tc.tile_pool(name="out1", bufs=4) as pout1,
            tc.tile_pool(name="one", bufs=1) as pone,
            tc.tile_pool(name="mm", bufs=4, space="PSUM") as pmm,
            tc.tile_pool(name="tr", bufs=2, space="PSUM") as ptr,
            tc.tile_pool(name="dram", bufs=4, space="DRAM") as pdr,
        ):
            # warmup collective fired immediately (input is an uninitialized
            # DRAM tile - AllGather/bypass only moves bytes): absorbs the
            # one-time ~50us cc-stream setup under the P1 matmul phase
            w_in = pdr.tile([P, 1], f32, tag="w_in", name="w_in")
            w_out = pdr.tile([NCORES, P, 1], f32, tag="w_out", name="w_out",
                             addr_space="Shared")
            nc.gpsimd.collective_compute(
                "AllGather", ALU.bypass, replica_groups=rg,
                ins=[w_in.opt()], outs=[w_out.opt()])

            ident = pone.tile([P, P], f32)
            make_identity(nc, ident[:])

            # ~3.5us of dummy matmuls while the first loads stream: tips the
            # PE's HAM activity window into the 2.4 GHz state before the real
            # work arrives (results exported so DCE keeps them)
            Pw = pmm.tile([P, P], f32, tag="WARM", name="Pw", bufs=1)
            for it in range(28):
                nc.tensor.matmul(Pw[:], ident[:], ident[:],
                                 start=(it == 0), stop=(it == 27))
            warm_sb = pone.tile([P, 2], f32)
            nc.vector.tensor_copy(out=warm_sb[:], in_=Pw[:, 0:2])
            nc.scalar.dma_start(out=warm_out.ap(), in_=warm_sb[:])

            i0s = pone.tile([P, NCH, D], bf16)
            nc.sync.dma_start(out=i0s[:], in_=i0s_in[:])
            u0s = pone.tile([P, NCH, D], bf16)
            nc.scalar.dma_start(out=u0s[:], in_=u0s_in[:])
            rud = pone.tile([P, nb, NB_CH], f32)
            nc.sync.dma_start(out=rud[:], in_=rud_in[:])
            rid = pone.tile([P, nb, NB_CH], f32)
            nc.sync.dma_start(out=rid[:], in_=rid_in[:])

            at_tiles, ac_tiles, state = {}, {}, {}

            def load(b):
                # At on the SP hwdge ring, Ac on the ACT ring: the two
                # physical HWDGE queues stream in parallel. Behavior 0's
                # quarters alternate across both rings so the very first P1
                # matmuls aren't load-starved.
                At = pbig.tile([P, NCH, BLK], fp8, tag="At", name=f"At{b}")
                Ac = pbig.tile([P, NCH, BLK], fp8, tag="Ac", name=f"Ac{b}")
                q = NCH // 4
                for g in range(4):
                    if b == 0:
                        eng_t = nc.sync if g % 2 == 0 else nc.scalar
                        eng_c = nc.scalar if g % 2 == 0 else nc.sync
                    else:
                        eng_t, eng_c = nc.sync, nc.scalar
                    eng_t.dma_start(out=At[:, g * q:(g + 1) * q, :],
                                    in_=At_in[b][:, g * q:(g + 1) * q, :])
                    eng_c.dma_start(out=Ac[:, g * q:(g + 1) * q, :],
                                    in_=Ac_in[b][:, g * q:(g + 1) * q, :])
                at_tiles[b], ac_tiles[b] = At, Ac

            def phase1(b):
                At, Ac = at_tiles[b], ac_tiles[b]
                # ---- P1u: u1_un.T = i0.T @ At ; P1i: i1_un.T = u0.T @ Ac
                Pu = pmm.tile([D, BLK], f32, tag="PC", name=f"P1u_{b}")
                for c in range(NCH):
                    nc.tensor.matmul(Pu[:], i0s[:, c, :], At[:, c, :],
                                     start=(c == 0), stop=(c == NCH - 1))
                Su = psm.tile([D, BLK], f32, tag="S", name=f"Su{b}")
                nc.vector.tensor_copy(out=Su[:], in_=Pu[:])

                Pi = pmm.tile([D, BLK], f32, tag="PC", name=f"P1i_{b}")
                for c in range(NCH):
                    nc.tensor.matmul(Pi[:], u0s[:, c, :], Ac[:, c, :],
                                     start=(c == 0), stop=(c == NCH - 1))
                Si = psm.tile([D, BLK], f32, tag="S", name=f"Si{b}")
                nc.vector.tensor_copy(out=Si[:], in_=Pi[:])

                # ---- transpose to natural layout + deg scale
                PT = ptr.tile([P, 2 * NB_CH, D], f32, tag="PT", name=f"PT{b}")
                for f in range(NB_CH):
                    nc.tensor.transpose(PT[:, f, :],
                                        Su[:, f * P:(f + 1) * P],
                                        ident[0:D, 0:D])
                for f in range(NB_CH):
                    nc.tensor.transpose(PT[:, NB_CH + f, :],
                                        Si[:, f * P:(f + 1) * P],
                                        ident[0:D, 0:D])
                u1s = pout1.tile([P, NB_CH, D], f32, tag="u1s", name=f"u1s{b}")
                i1s = pout1.tile([P, NB_CH, D], f32, tag="i1s", name=f"i1s{b}")
                zi = psm.tile([P, NB_CH, 2 * D], bf16, tag="zi", name=f"zi{b}")
                for f in range(NB_CH):
                    nc.vector.tensor_scalar_mul(out=u1s[:, f, :],
                                                in0=PT[:, f, :],
                                                scalar1=rud[:, b, f:f + 1])
                    nc.vector.tensor_scalar_mul(out=i1s[:, f, :],
                                                in0=PT[:, NB_CH + f, :],
                                                scalar1=rid[:, b, f:f + 1])
                nc.vector.tensor_copy(out=zi[:, :, 0:D], in_=u1s[:])
                nc.vector.tensor_copy(out=zi[:, :, D:2 * D], in_=i1s[:])
                nc.scalar.dma_start(out=u1_out[b].ap(), in_=u1s[:])
                nc.scalar.dma_start(out=i1_out[b].ap(), in_=i1s[:])

                # write this behavior's payload into its group's AG input
                gi, v = grp_of[b]
                z_in = zin_groups[gi]
                nc.scalar.dma_start(out=z_in[:, v, :, :], in_=zi[:])
                state[b] = None

            def gather_group(gi):
                # batched AllGather: front group carries nb-1 behaviors, the
                # last behavior gathers alone, so only one 14us P2 group
                # serializes behind the final (small) collective while the
                # per-op ~20us TOPSP latency is paid as few times as possible
                members = groups[gi]
                z_in = zin_groups[gi]
                z_out = pdr.tile([NCORES, P, len(members), NB_CH, 2 * D],
                                 bf16, tag=f"z_out{gi}", name=f"z_out{gi}",
                                 addr_space="Shared")
                nc.gpsimd.collective_compute(
                    "AllGather", ALU.bypass, replica_groups=rg,
                    ins=[z_in.opt()], outs=[z_out.opt()])
                for v, b in enumerate(members):
                    G = pgat.tile([P, NCORES, NB_CH, 2 * D], bf16, tag="G",
                                  name=f"G{b}")
                    nc.sync.dma_start(
                        out=G[:],
                        in_=z_out[:, :, v, :, :].rearrange(
                            "c p f x -> p c f x"))
                    state[b] = G

            def phase2(b):
                At, Ac = at_tiles.pop(b), ac_tiles.pop(b)
                G = state.pop(b)
                # ---- P2u: u2_un.T = i1_full.T @ At
                Pu = pmm.tile([D, BLK], f32, tag="PC", name=f"P2u_{b}")
                for c in range(NCH):
                    nc.tensor.matmul(Pu[:], G[:, c // NB_CH, c % NB_CH, D:2 * D],
                                     At[:, c, :],
                                     start=(c == 0), stop=(c == NCH - 1))
                Tu = psm.tile([D, BLK], f32, tag="T2", name=f"Tu{b}")
                nc.vector.tensor_copy(out=Tu[:], in_=Pu[:])
                nc.scalar.dma_start(out=u2_out[b].ap(), in_=Tu[:])
                # ---- P2i: i2_un.T = u1_full.T @ Ac
                Pi = pmm.tile([D, BLK], f32, tag="PC", name=f"P2i_{b}")
                for c in range(NCH):
                    nc.tensor.matmul(Pi[:], G[:, c // NB_CH, c % NB_CH, 0:D],
                                     Ac[:, c, :],
                                     start=(c == 0), stop=(c == NCH - 1))
                Ti = psm.tile([D, BLK], f32, tag="T2", name=f"Ti{b}")
                nc.vector.tensor_copy(out=Ti[:], in_=Pi[:])
                nc.scalar.dma_start(out=i2_out[b].ap(), in_=Ti[:])

            # all loads upfront (HWDGE rings drain in ~30us, long before the
            # gather readbacks queue behind them); all P1s before all P2s so
            # every AllGather hides under the remaining P1 matmul stream
            if nb >= 2:
                groups = [list(range(nb - 1)), [nb - 1]]
            else:
                groups = [list(range(nb))]
            grp_of = {}
            for gi, members in enumerate(groups):
                for v, b in enumerate(members):
                    grp_of[b] = (gi, v)
            zin_groups = [
                pdr.tile([P, len(members), NB_CH, 2 * D], bf16,
                         tag=f"z_in{gi}", name=f"z_in{gi}")
                for gi, members in enumerate(groups)]
            last_of_group = {members[-1]: gi for gi, members in
                             enumerate(groups)}
            for b in range(nb):
                load(b)
            for b in range(nb):
                phase1(b)
                if b in last_of_group:
                    gather_group(last_of_group[b])
            for b in range(nb):
                phase2(b)

    nc.compile()
    return nc


# --------------------------------------------------------------------------
# host-side helpers
# --------------------------------------------------------------------------

def _chunk_part(x):
    """[4096, C] -> [128, 32, C] with row = c*128 + p."""
    return np.ascontiguousarray(
        x.reshape(NCH, P, x.shape[1]).transpose(1, 0, 2))


def host_prep_behavior(R):
    """Quantize to e3m4 + exact f64 degree reciprocals."""
    Rq = R.astype(_E3M4)
    deg_u = R.sum(axis=1, dtype=np.float64)
    deg_i = R.sum(axis=0, dtype=np.float64)
    ru = (1.0 / (deg_u + EPS)).astype(np.float32)
    ri = (1.0 / (deg_i + EPS)).astype(np.float32)
    return Rq, ru, ri, deg_u, deg_i


def _chunk_order(k):
    """Gather slots arrive in rank order: identity chunk order."""
    return list(range(NCH))


def _core_layouts(Rq, k, order):
    """Per-core At/Ac tiles in [128, 32, 512] linear-DMA order, with the
    32 contraction chunks permuted to the gather slot order."""
    rows = Rq[k * BLK:(k + 1) * BLK, :]          # [512, 4096]
    # At[p, ic, u] = rows[u, ic*128+p]
    At = np.ascontiguousarray(
        rows.T.reshape(NCH, P, BLK)[order].transpose(1, 0, 2))
    cols = Rq[:, k * BLK:(k + 1) * BLK]          # [4096, 512]
    # Ac[p, uc, j] = cols[uc*128+p, j]
    Ac = np.ascontiguousarray(
        cols.reshape(NCH, P, BLK)[order].transpose(1, 0, 2))
    return At, Ac


def prep_in_maps(prepped, u0, i0):
    i0s = _chunk_part(i0.astype(_BF16))
    u0s = _chunk_part(u0.astype(_BF16))
    in_maps = []
    for k in range(NCORES):
        sl = slice(k * BLK, (k + 1) * BLK)
        order = _chunk_order(k)
        m = {"i0s": np.ascontiguousarray(i0s[:, order, :]),
             "u0s": np.ascontiguousarray(u0s[:, order, :])}
        # rud[p, b, f] = 1/deg_u[k*512 + f*128 + p] for behavior b
        m["rud"] = np.ascontiguousarray(np.stack(
            [p[1][sl].reshape(NB_CH, P).T for p in prepped], axis=1))
        m["rid"] = np.ascontiguousarray(np.stack(
            [p[2][sl].reshape(NB_CH, P).T for p in prepped], axis=1))
        for b, p in enumerate(prepped):
            At, Ac = _core_layouts(p[0], k, order)
            m[f"At{b}"] = At
            m[f"Ac{b}"] = Ac
        in_maps.append(m)
    return in_maps


def assemble_dense(results, prepped, nb):
    """Per-behavior (u_acc [N,D], i_acc [N,D]) from per-core outputs."""
    out = []
    for b in range(nb):
        _, ru, ri, _, _ = prepped[b]
        u_acc = np.empty((N, D), np.float32)
        i_acc = np.empty((N, D), np.float32)
        for k in range(NCORES):
            sl = slice(k * BLK, (k + 1) * BLK)
            u1 = results[k][f"u1_{b}"].transpose(1, 0, 2).reshape(BLK, D)
            i1 = results[k][f"i1_{b}"].transpose(1, 0, 2).reshape(BLK, D)
            u2 = results[k][f"u2_{b}"].T * ru[sl][:, None]
            i2 = results[k][f"i2_{b}"].T * ri[sl][:, None]
            u_acc[sl] = (u1 + u2) * np.float32(0.5)
            i_acc[sl] = (i1 + i2) * np.float32(0.5)
        out.append((u_acc, i_acc))
    return out


def ones_behavior(u0, i0):
    """Analytic LightGCN-2-layer outputs when R is all-ones [N, N]."""
    s_i = i0.astype(np.float64).sum(axis=0)
    s_u = u0.astype(np.float64).sum(axis=0)
    d = N + EPS
    u_row = (s_i / d + s_u * N / (d * d)) * 0.5
    i_row = (s_u / d + s_i * N / (d * d)) * 0.5
    u = np.broadcast_to(u_row.astype(np.float32), (N, D)).copy()
    it = np.broadcast_to(i_row.astype(np.float32), (N, D)).copy()
    return u, it


# --------------------------------------------------------------------------
# cached device runner (compile once per behavior-count, run many)
# --------------------------------------------------------------------------

_RUNNERS = {}


class _Runner:
    def __init__(self, nb):
        self.nb = nb
        self.nc = build_program(nb)
        self._jitted = None
        self._meta = None

    def _prep_jit(self):
        import jax
        import numpy as _np
        from jax.sharding import Mesh, PartitionSpec
        from jax.experimental.shard_map import shard_map
        from concourse import bass2jax
        from concourse.bass2jax import _bass_exec_p, partition_id_tensor
        import concourse.mybir as mybir

        bass2jax.install_neuronx_cc_hook()
        nc = self.nc
        partition_name = (nc.partition_id_tensor.name
                          if nc.partition_id_tensor else None)
        in_names, out_names, out_avals, zero_shapes = [], [], [], []
        for alloc in nc.m.functions[0].allocations:
            if not isinstance(alloc, mybir.MemoryLocationSet):
                continue
            name = alloc.memorylocations[0].name
            if alloc.kind == "ExternalInput":
                if name != partition_name:
                    in_names.append(name)
            elif alloc.kind == "ExternalOutput":
                shape = tuple(alloc.tensor_shape)
                dtype = mybir.dt.np(alloc.dtype)
                out_names.append(name)
                out_avals.append(jax.core.ShapedArray(shape, dtype))
                zero_shapes.append((shape, dtype))
        n_params = len(in_names)
        full_in_names = list(in_names) + list(out_names)
        if partition_name is not None:
            full_in_names.append(partition_name)

        def _body(*args):
            operands = list(args)
            if partition_name is not None:
                operands.append(partition_id_tensor())
            outs = _bass_exec_p.bind(
                *operands,
                out_avals=tuple(out_avals),
                in_names=tuple(full_in_names),
                out_names=tuple(out_names),
                lowering_input_output_aliases=(),
                sim_require_finite=True,
                sim_require_nnan=True,
                nc=nc,
            )
            return tuple(outs)

        devices = jax.devices()[:NCORES]
        mesh = Mesh(_np.asarray(devices), ("core",))
        n_outs = len(out_names)
        in_specs = (PartitionSpec("core"),) * (n_params + n_outs)
        out_specs = (PartitionSpec("core"),) * n_outs
        donate = tuple(range(n_params, n_params + n_outs))
        self._jitted = jax.jit(
            shard_map(_body, mesh=mesh, in_specs=in_specs,
                      out_specs=out_specs, check_rep=False),
            donate_argnums=donate, keep_unused=True)
        self._meta = (in_names, out_names, out_avals, zero_shapes, n_params)

    def run(self, in_maps):
        if self._jitted is None:
            self._prep_jit()
        import numpy as _np
        in_names, out_names, out_avals, zero_shapes, n_params = self._meta
        concat_in = [
            _np.concatenate([_np.asarray(in_maps[c][nm]) for c in range(NCORES)],
                            axis=0)
            for nm in in_names]
        concat_zeros = [_np.zeros((NCORES * s[0], *s[1:]), dt)
                        for (s, dt) in zero_shapes]
        out_arrs = self._jitted(*concat_in, *concat_zeros)
        results = []
        for c in range(NCORES):
            results.append({
                nm: _np.asarray(out_arrs[i]).reshape(
                    NCORES, *out_avals[i].shape)[c]
                for i, nm in enumerate(out_names)})
        return results

    def run_traced(self, in_maps, tmpdir=None):
        """Run through run_bass_kernel_spmd with NTFF tracing (recompiles)."""
        _install_trace_shims()
        from concourse.bass_utils import run_bass_kernel_spmd
        return run_bass_kernel_spmd(self.nc, in_maps,
                                    core_ids=list(range(NCORES)),
                                    trace=True, tmpdir=tmpdir)


def _install_trace_shims():
    """This image's antenv lacks axon_hooks (the NTFF-hook registry) and has
    no artifact bucket; recreate the hook from the boot recipe and make
    artifact upload a local no-op."""
    import sys, types, importlib.util

    if "antenv.axon_hooks" not in sys.modules:
        mod = types.ModuleType("antenv.axon_hooks")
        mod._hook = None

        def set_axon_ntff_profile_hook(h):
            mod._hook = h

        def get_axon_ntff_profile_hook():
            return mod._hook

        mod.set_axon_ntff_profile_hook = set_axon_ntff_profile_hook
        mod.get_axon_ntff_profile_hook = get_axon_ntff_profile_hook
        import antenv
        sys.modules["antenv.axon_hooks"] = mod
        antenv.axon_hooks = mod

        spec = importlib.util.spec_from_file_location(
            "trn_boot_shim", "/root/.axon_site/trn_agent_boot/trn_boot.py")
        boot = importlib.util.module_from_spec(spec)
        spec.loader.exec_module(boot)
        hook = boot._ntff_profile_via_ctypes("/opt/axon/libaxon_pjrt.so")
        mod._hook = hook

    import concourse.bass_utils as bu
    if not getattr(bu.upload_artifacts, "_is_local_shim", False):
        def _local_upload(tmpdir):
            return tmpdir
        _local_upload._is_local_shim = True
        bu.upload_artifacts = _local_upload


def get_runner(nb):
    if nb not in _RUNNERS:
        _RUNNERS[nb] = _Runner(nb)
    return _RUNNERS[nb]


# --------------------------------------------------------------------------
# entry point
# --------------------------------------------------------------------------

def _is_ones(a):
    return a[0, 0] == 1.0 and bool(np.all(a == np.float32(1.0)))


def kernel(**inputs):
    inputs = {k: np.asarray(v) for k, v in inputs.items()}
    u0 = np.ascontiguousarray(inputs["user_embedding"], dtype=np.float32)
    i0 = np.ascontiguousarray(inputs["item_embedding"], dtype=np.float32)

    real_names = ["R_click", "R_fav", "R_cart", "R_buy"]
    virt_names = [("M_click", "add_click"), ("M_fav", "add_fav"),
                  ("M_cart", "add_cart")]
    mats = [np.asarray(inputs[n], dtype=np.float32) for n in real_names]
    mats += [np.asarray(inputs[m], dtype=np.float32) for m, _ in virt_names]

    dense_idx = [j for j, a in enumerate(mats) if not _is_ones(a)]
    per_behavior = [None] * 7

    if dense_idx:
        nb = len(dense_idx)
        runner = get_runner(nb)
        prepped = [host_prep_behavior(mats[j]) for j in dense_idx]
        in_maps = prep_in_maps(prepped, u0, i0)
        results = runner.run(in_maps)
        dense = assemble_dense(results, prepped, nb)
        for pos, j in enumerate(dense_idx):
            per_behavior[j] = dense[pos]

    ones_cache = None
    for j, a in enumerate(mats):
        if per_behavior[j] is None:
            if ones_cache is None:
                ones_cache = ones_behavior(u0, i0)
            per_behavior[j] = ones_cache

    ur = [per_behavior[j][0] for j in range(4)]
    ir = [per_behavior[j][1] for j in range(4)]
    uv = [per_behavior[4 + j][0] + np.asarray(inputs[virt_names[j][1]],
                                              dtype=np.float32)
          for j in range(3)]
    iv = [per_behavior[4 + j][1] for j in range(3)]

    out = np.concatenate(
        [np.stack(ur), np.stack(ir), np.stack(uv), np.stack(iv)], axis=0)
    return np.ascontiguousarray(out, dtype=np.float32)

